# revision 1
# baseline (speedup 1.0000x reference)
"""Causal attention (flattened-head GQA variant) for TRN2, 8 NeuronCores.

Problem structure exploited:
  - K/V are group-projections tiled 4x along the head dim, and the score
    contraction runs over the full flattened 1024 dim.  Algebraically:
        att = Q @ tile(Kg,4)^T = (sum of Q's four 256-col blocks) @ Kg^T
        out = att_sm @ tile(Vg,4) = tile(att_sm @ Vg, 4)
    so the device only computes with 256-wide Qsum/Kg/Vg.
  - Softmax needs no max-subtraction here (logits bounded ~60; exp fits fp32
    comfortably), so scores are computed directly in the transposed layout
    U^T[s,t] = exp(Kg @ Qsum^T) and fed straight into the AV matmul as the
    stationary operand -- no on-device transposes at all.
  - Row sums come from a ones-column appended to Vg (PSUM col 256).
  - Block-causal skipping: s-tiles entirely above the diagonal are never
    computed; diagonal 128x256 blocks are masked with precomputed 0/1 tiles.
  - Fused chunk loop: for each 256-wide t-chunk, DMA x, project Q/K for the
    chunk, project V for its two s-tiles, then compute score block J=chunk
    (causally needs only chunks <= J) and its AV output.  This fills the
    DMA-paced load phase with score/AV compute and keeps the PE saturated.

Precision: QK path in fp16 (11-bit mantissa, full PE rate, half DMA), scores
accumulated in fp32 PSUM, exp/AV path in bf16 (needs bf16's exponent range:
unnormalized exp values reach ~1e26).  End-to-end absmax rel error vs the
fp32 reference ~5e-3.

Sharding: data-parallel over batch B=8, one batch per core, no collectives.
"""

import os
import numpy as np
import ml_dtypes
from contextlib import ExitStack

import concourse.tile as tile
from concourse import bacc, mybir
from concourse.bass_utils import run_bass_kernel_spmd

B, T, D = 8, 2048, 1024
C = 256          # group width (N_QUERY_GROUPS * HEAD_SIZE)
P = 128
ND = D // P      # 8 contraction tiles for projections
NS = T // P      # 16 s-tiles
JB = 256         # t-chunk width
NJB = T // JB    # 8
NCORES = 8

F32 = mybir.dt.float32
FP16 = mybir.dt.float16
BF16 = mybir.dt.bfloat16


def _build():
    nc = bacc.Bacc("TRN2", target_bir_lowering=False, debug=False)
    xT = nc.dram_tensor("xT", [D, T], FP16, kind="ExternalInput").ap()
    wq = nc.dram_tensor("wq", [D, C], FP16, kind="ExternalInput").ap()
    wk = nc.dram_tensor("wk", [D, C], FP16, kind="ExternalInput").ap()
    wv = nc.dram_tensor("wv", [D, C], FP16, kind="ExternalInput").ap()
    bqk = nc.dram_tensor("bqk", [P, 4], F32, kind="ExternalInput").ap()
    bvb = nc.dram_tensor("bvb", [P, C], FP16, kind="ExternalInput").ap()
    msk = nc.dram_tensor("msk", [P, 2, JB], mybir.dt.float8e4, kind="ExternalInput").ap()
    o = nc.dram_tensor("o", [T, C], F32, kind="ExternalOutput").ap()

    with tile.TileContext(nc) as tc, ExitStack() as ctx:
        cst = ctx.enter_context(tc.tile_pool(name="cst", bufs=1))
        big = ctx.enter_context(tc.tile_pool(name="big", bufs=1))
        up = ctx.enter_context(tc.tile_pool(name="up", bufs=3))
        outp = ctx.enter_context(tc.tile_pool(name="outp", bufs=3))
        pp = ctx.enter_context(tc.tile_pool(name="pp", bufs=2, space="PSUM"))
        pst = ctx.enter_context(tc.tile_pool(name="pst", bufs=4, space="PSUM"))
        pav = ctx.enter_context(tc.tile_pool(name="pav", bufs=2, space="PSUM"))

        bqk_t = cst.tile([P, 4], F32, tag="bqk")
        nc.sync.dma_start(bqk_t[:], bqk)
        bvb_t = cst.tile([P, C], FP16, tag="bvb")
        msk_t = cst.tile([P, 2, JB], mybir.dt.float8e4, tag="msk")

        wr = {}
        for _n in ("q", "k", "v"):
            wr[_n] = cst.tile([P, ND, C], FP16, tag=f"w{_n}", name=f"wr_{_n}")

        # PE warm-up: ~5us of matmuls on a zeroed scratch tile while the
        # first DMAs land -- fills the idle start window and finishes the
        # PE clock ramp (HAM) before real work arrives.
        wrm = cst.tile([P, C], FP16, tag="wrm")
        nc.vector.memset(wrm[:], 0.0)
        for wi in range(16):
            ps_w = pp.tile([P, 2 * JB], F32, tag="pp", name=f"warm_{wi}")
            nc.tensor.matmul(ps_w[:, :C], wrm[:, :P], wrm[:], start=True, stop=True)

        xtr = big.tile([P, ND, T], FP16, tag="xtr")
        qkT = {"q": big.tile([P, 2, T], FP16, tag="qsT", name="qsT"),
               "k": big.tile([P, 2, T], FP16, tag="ksT", name="ksT")}
        vg = big.tile([P, NS, C + 1], BF16, tag="vg")
        nc.vector.memset(vg[:, :, C:C + 1], 8.0)

        uts = {}

        def do_st(J):
            # scores^T -> exp for t-block J
            jt = slice(J * JB, (J + 1) * JB)
            ut = up.tile([P, NS, JB], BF16, tag="ut", name=f"ut_{J}")
            uts[J] = ut
            for sp in range(J + 1):
                si0 = 2 * sp
                ps_t = pst.tile([P, 2 * JB], F32, tag="pst",
                                name=f"pst_{J}_{sp}")
                if sp < J:
                    for h in range(2):
                        si = si0 + h
                        for ct in range(2):
                            nc.tensor.matmul(
                                ps_t[:, h * JB:(h + 1) * JB],
                                qkT["k"][:, ct, si * P:(si + 1) * P],
                                qkT["q"][:, ct, jt],
                                start=(ct == 0), stop=(ct == 1),
                            )
                    nc.scalar.activation(ut[:, si0:si0 + 2, :], ps_t[:],
                                         mybir.ActivationFunctionType.Exp)
                else:
                    # diagonal pair: si0 needs all 256 t-cols; si0+1 only
                    # its second 128 (AV q=0 never reads si0+1) -> N=128
                    for ct in range(2):
                        nc.tensor.matmul(
                            ps_t[:, 0:JB],
                            qkT["k"][:, ct, si0 * P:(si0 + 1) * P],
                            qkT["q"][:, ct, jt],
                            start=(ct == 0), stop=(ct == 1),
                        )
                    for ct in range(2):
                        nc.tensor.matmul(
                            ps_t[:, JB:JB + P],
                            qkT["k"][:, ct, (si0 + 1) * P:(si0 + 2) * P],
                            qkT["q"][:, ct, J * JB + P:(J + 1) * JB],
                            start=(ct == 0), stop=(ct == 1),
                        )
                    nc.scalar.activation(ut[:, si0, :], ps_t[:, 0:JB],
                                         mybir.ActivationFunctionType.Exp)
                    nc.scalar.activation(ut[:, si0 + 1, P:JB],
                                         ps_t[:, JB:JB + P],
                                         mybir.ActivationFunctionType.Exp)
                    nc.vector.tensor_tensor(ut[:, si0, :], ut[:, si0, :],
                                            msk_t[:, 0, :],
                                            mybir.AluOpType.mult)
                    nc.vector.tensor_tensor(ut[:, si0 + 1, P:JB],
                                            ut[:, si0 + 1, P:JB],
                                            msk_t[:, 1, P:JB],
                                            mybir.AluOpType.mult)

        def do_v(tb):
            # V projection for chunk tb's two s-tiles
            for si in (2 * tb, 2 * tb + 1):
                pv = pav.tile([P, C + 1], F32, tag="pav", name=f"pv_{si}")[:, :JB]
                for d in range(ND):
                    nc.tensor.matmul(
                        pv,
                        xtr[:, d, si * P:(si + 1) * P],
                        wr["v"][:, d, :],
                        start=(d == 0), stop=(d == ND - 1),
                    )
                nc.vector.tensor_tensor(vg[:, si, :C], pv, bvb_t[:],
                                        mybir.AluOpType.add)

        def do_av(J):
            ut = uts[J]
            for q in range(2):
                tci = 2 * J + q
                pa = pav.tile([P, C + 1], F32, tag="pav", name=f"pav_{J}_{q}")
                for si in range(tci + 1):
                    nc.tensor.matmul(
                        pa[:],
                        ut[:, si, q * P:(q + 1) * P],
                        vg[:, si, :],
                        start=(si == 0), stop=(si == tci),
                    )
                recip = outp.tile([P, 1], F32, tag="recip")
                nc.vector.reciprocal(recip[:], pa[:, C:C + 1])
                ob = outp.tile([P, C], F32, tag="ob")
                nc.vector.tensor_scalar_mul(ob[:], pa[:, :C], recip[:])
                nc.sync.dma_start(o[tci * P:(tci + 1) * P, :], ob[:])

        def do_proj(psl):
            pw = psl.stop - psl.start
            for mi, mat in enumerate(("q", "k")):
                for ct in range(2):
                    ps_p = pp.tile([P, 2 * JB], F32, tag="pp",
                                   name=f"pp_{mat}{ct}_{psl.start}")[:, :pw]
                    for d in range(ND):
                        nc.tensor.matmul(
                            ps_p,
                            wr[mat][:, d, ct * P:(ct + 1) * P],
                            xtr[:, d, psl],
                            start=(d == 0), stop=(d == ND - 1),
                        )
                    nc.vector.tensor_scalar_add(
                        qkT[mat][:, ct, psl],
                        ps_p,
                        bqk_t[:, 2 * mi + ct: 2 * mi + ct + 1],
                    )

        # ---- pair 0: startup DMAs, projections, scores only (V/AV for
        # chunks 0-1 are deferred into pair 1 so the DMA queue delivers
        # x chunks 2-3 before the V weights) ----
        ts0 = slice(0, JB)
        xsrc0 = xT[:, ts0].rearrange("(o p) t -> p o t", p=P)
        wq_src = wq.rearrange("(o p) c -> p o c", p=P)
        nc.sync.dma_start(wr["q"][:, :, 0:P], wq_src[:, :, 0:P])
        nc.sync.dma_start(xtr[:, 0:2, ts0], xsrc0[:, 0:2, :])
        nc.sync.dma_start(xtr[:, 2:4, ts0], xsrc0[:, 2:4, :])
        nc.sync.dma_start(wr["q"][:, :, P:C], wq_src[:, :, P:C])
        nc.sync.dma_start(xtr[:, 4:6, ts0], xsrc0[:, 4:6, :])
        nc.sync.dma_start(xtr[:, 6:8, ts0], xsrc0[:, 6:8, :])
        nc.sync.dma_start(wr["k"][:], wk.rearrange("(o p) c -> p o c", p=P))
        ts1 = slice(JB, 2 * JB)
        nc.sync.dma_start(xtr[:, :, ts1],
                          xT[:, ts1].rearrange("(o p) t -> p o t", p=P))
        nc.sync.dma_start(msk_t[:], msk)
        nc.sync.dma_start(bvb_t[:], bvb)
        do_proj(ts0)
        do_proj(ts1)
        do_st(0)
        do_st(1)

        # ---- pair 1: x chunks 2-3 queue ahead of the V weights; then the
        # deferred V/AV for chunks 0-1 (emitted after the wv DMA so the RAW
        # dependency is tracked and properly semaphore-guarded) ----
        pts = slice(2 * JB, 4 * JB)
        nc.sync.dma_start(xtr[:, :, pts],
                          xT[:, pts].rearrange("(o p) t -> p o t", p=P))
        nc.sync.dma_start(wr["v"][:], wv.rearrange("(o p) c -> p o c", p=P))
        do_v(0)
        do_av(0)
        do_v(1)
        do_av(1)
        do_proj(pts)
        for tb in (2, 3):
            do_st(tb)
            do_v(tb)
            do_av(tb)

        # ---- pairs 2-3 ----
        for pb in (2, 3):
            tb0 = 2 * pb
            pts = slice(tb0 * JB, (tb0 + 2) * JB)
            nc.sync.dma_start(xtr[:, :, pts],
                              xT[:, pts].rearrange("(o p) t -> p o t", p=P))
            do_proj(pts)
            for tb in (tb0, tb0 + 1):
                do_st(tb)
                do_v(tb)
                do_av(tb)

    nc.compile()
    return nc


_CACHE = {}
LAST_EXEC_TIME_NS = None


def _get_nc():
    if "nc" not in _CACHE:
        _CACHE["nc"] = _build()
    return _CACHE["nc"]


def kernel(x, Wq, bq, Wk, bk, Wv, bv):
    x = np.asarray(x, dtype=np.float32)
    Wq = np.asarray(Wq, dtype=np.float32)
    bq = np.asarray(bq, dtype=np.float32)
    Wk = np.asarray(Wk, dtype=np.float32)
    bk = np.asarray(bk, dtype=np.float32)
    Wv = np.asarray(Wv, dtype=np.float32)
    bv = np.asarray(bv, dtype=np.float32)

    # Fold the 4x head-tiling into the weights: contraction with tile(Kg,4)
    # equals contraction of block-summed Q with Kg.
    wq_s = Wq.reshape(D, 4, C).sum(axis=1, dtype=np.float64).astype(np.float32)
    bq_s = bq.reshape(4, C).sum(axis=0, dtype=np.float64).astype(np.float32)

    bqk = np.stack([bq_s[:P], bq_s[P:], bk[:P], bk[P:]], axis=1).astype(np.float32)
    bvb = np.broadcast_to(bv, (P, C)).astype(np.float32)

    # Diagonal-block causal masks: keep t >= s  <=>  j >= 128*m + p.
    jj = np.arange(JB)[None, None, :]
    pp_ = np.arange(P)[:, None, None]
    mm = np.arange(2)[None, :, None]
    msk = (jj >= P * mm + pp_).astype(ml_dtypes.float8_e4m3)

    shared = {
        "wq": np.ascontiguousarray(wq_s.astype(np.float16)),
        "wk": np.ascontiguousarray(Wk.astype(np.float16)),
        "wv": np.ascontiguousarray(Wv.astype(np.float16)),
        "bqk": np.ascontiguousarray(bqk), "bvb": np.ascontiguousarray(bvb.astype(np.float16)),
        "msk": np.ascontiguousarray(msk),
    }
    in_maps = []
    for b in range(B):
        m = dict(shared)
        m["xT"] = np.ascontiguousarray(x[b].T.astype(np.float16))
        in_maps.append(m)

    nc = _get_nc()
    try:
        res = run_bass_kernel_spmd(nc, in_maps, core_ids=list(range(NCORES)))
    except ModuleNotFoundError:
        # BASS_TRACE=1 requests NTFF profiling, but this container type has
        # no axon NTFF hook (antenv.axon_hooks absent) -- rerun untraced.
        os.environ["BASS_NEVER_TRACE"] = "1"
        res = run_bass_kernel_spmd(nc, in_maps, core_ids=list(range(NCORES)))
    global LAST_EXEC_TIME_NS
    LAST_EXEC_TIME_NS = res.exec_time_ns
    if res.exec_time_ns is not None:
        print(f"HW exec time: {res.exec_time_ns} ns")

    out = np.empty((1, B, T, 4 * C), dtype=np.float32)
    for b in range(B):
        ob = res.results[b]["o"]
        out[0, b] = np.tile(ob, (1, 4))
    return out



# revision 4
# speedup vs baseline: 1.0653x; 1.0653x over previous
"""Causal attention (flattened-head GQA variant) for TRN2, 8 NeuronCores.

Problem structure exploited:
  - K/V are group-projections tiled 4x along the head dim, and the score
    contraction runs over the full flattened 1024 dim.  Algebraically:
        att = Q @ tile(Kg,4)^T = (sum of Q's four 256-col blocks) @ Kg^T
        out = att_sm @ tile(Vg,4) = tile(att_sm @ Vg, 4)
    so the device only computes with 256-wide Qsum/Kg/Vg.
  - Projections run in fp8e4m3 DoubleRow mode (PE contracts 256 rows/pass at
    0.5 cycles/row = 4x fp16 rate) with error compensation: host splits
    x ~ x1 + x2 and W ~ W1 + W2 (each fp8, residual split), device computes
    x1W1 + x1W2 + x2W1 in one PSUM group (12 DR matmuls vs 16 fp16-equiv
    passes).  Dropped x2W2 term ~2^-8 relative.  Operands are pre-scaled by
    powers of 2 (x: 32, W: 4096 / 1024 for the Q block-sum) to sit in e4m3's
    normal range; the PSUM scale (2^15 Q / 2^17 K,V) is absorbed by the fused
    scale+bias tensor_scalar for Q/K, and for V rides into vg where it
    cancels against the ones-column (8*2^17) in the rowsum normalization.
  - Softmax needs no max-subtraction (logits bounded ~60; exp fits fp32),
    so scores are computed directly in the transposed layout
    U^T[s,t] = exp(Kg @ Qsum^T) and fed straight into the AV matmul as the
    stationary operand -- no on-device transposes at all.
  - Block-causal skipping: s-tiles entirely above the diagonal are never
    computed; diagonal 128x256 blocks are masked with precomputed 0/1 tiles.
  - Fused chunk loop keeps PE saturated while DMA streams x chunks.

Precision: fp8-3-term projections (~2^-8 rel), score matmul fp16, exp/AV
path bf16.  End-to-end absmax rel error vs fp32 reference ~1.75e-2.

Sharding: data-parallel over batch B=8, one batch per core, no collectives.
"""

import os
import numpy as np
import ml_dtypes
from contextlib import ExitStack

import concourse.tile as tile
from concourse import bacc, mybir
from concourse.bass_utils import run_bass_kernel_spmd

B, T, D = 8, 2048, 1024
C = 256          # group width (N_QUERY_GROUPS * HEAD_SIZE)
P = 128
ND = D // P      # 8 contraction tiles for projections
NDP = ND // 2    # 4 DoubleRow contraction pairs
NS = T // P      # 16 s-tiles
JB = 256         # t-chunk width
NJB = T // JB    # 8
NCORES = 8

F32 = mybir.dt.float32
FP16 = mybir.dt.float16
BF16 = mybir.dt.bfloat16
FP8 = mybir.dt.float8e4
DR = mybir.MatmulPerfMode.DoubleRow

SX = 32.0        # x pre-scale into e4m3 range
SWQ = 1024.0     # wq_s pre-scale (block-summed W spans +-0.125)
SWK = 4096.0     # wk/wv pre-scale (+-1/32)
QSC = 1.0 / (SX * SWQ)   # PSUM -> Q descale
KSC = 1.0 / (SX * SWK)   # PSUM -> K descale
VSC = SX * SWK           # V path stays scaled; ones-col = 8*VSC


def _build():
    nc = bacc.Bacc("TRN2", target_bir_lowering=False, debug=False)
    x1T = nc.dram_tensor("x1T", [D, T], FP8, kind="ExternalInput").ap()
    x2T = nc.dram_tensor("x2T", [D, T], FP8, kind="ExternalInput").ap()
    wsrc = {}
    for _n in ("q", "k", "v"):
        for _h in (1, 2):
            wsrc[_n, _h] = nc.dram_tensor(f"w{_n}{_h}", [D, C], FP8,
                                          kind="ExternalInput").ap()
    bqk = nc.dram_tensor("bqk", [P, 4], F32, kind="ExternalInput").ap()
    bvb = nc.dram_tensor("bvb", [P, C], FP16, kind="ExternalInput").ap()
    msk = nc.dram_tensor("msk", [P, 2, JB], mybir.dt.float8e4, kind="ExternalInput").ap()
    o = nc.dram_tensor("o", [T, C], F32, kind="ExternalOutput").ap()

    with tile.TileContext(nc) as tc, ExitStack() as ctx:
        cst = ctx.enter_context(tc.tile_pool(name="cst", bufs=1))
        big = ctx.enter_context(tc.tile_pool(name="big", bufs=1))
        up = ctx.enter_context(tc.tile_pool(name="up", bufs=3))
        outp = ctx.enter_context(tc.tile_pool(name="outp", bufs=3))
        pp = ctx.enter_context(tc.tile_pool(name="pp", bufs=2, space="PSUM"))
        pst = ctx.enter_context(tc.tile_pool(name="pst", bufs=4, space="PSUM"))
        pav = ctx.enter_context(tc.tile_pool(name="pav", bufs=2, space="PSUM"))

        bqk_t = cst.tile([P, 4], F32, tag="bqk")
        nc.sync.dma_start(bqk_t[:], bqk)
        bvb_t = cst.tile([P, C], FP16, tag="bvb")
        msk_t = cst.tile([P, 2, JB], mybir.dt.float8e4, tag="msk")

        wr = {}
        for _n in ("q", "k", "v"):
            for _h in (1, 2):
                wr[_n, _h] = cst.tile([P, ND, C], FP8, tag=f"w{_n}{_h}",
                                      name=f"wr_{_n}{_h}")

        # PE warm-up: ~5us of matmuls on a zeroed scratch tile while the
        # first DMAs land -- fills the idle start window and finishes the
        # PE clock ramp (HAM) before real work arrives.
        wrm = cst.tile([P, C], FP16, tag="wrm")
        nc.vector.memset(wrm[:], 0.0)
        for wi in range(16):
            ps_w = pp.tile([P, 2 * JB], F32, tag="pp", name=f"warm_{wi}")
            nc.tensor.matmul(ps_w[:, :C], wrm[:, :P], wrm[:], start=True, stop=True)

        xtr1 = big.tile([P, ND, T], FP8, tag="xtr1")
        xtr2 = big.tile([P, ND, T], FP8, tag="xtr2")
        qkT = {"q": big.tile([P, 2, T], FP16, tag="qsT", name="qsT"),
               "k": big.tile([P, 2, T], FP16, tag="ksT", name="ksT")}
        vg = big.tile([P, NS, C + 1], BF16, tag="vg")
        nc.vector.memset(vg[:, :, C:C + 1], 8.0 * VSC)

        uts = {}

        def do_st(J):
            # scores^T -> exp for t-block J
            jt = slice(J * JB, (J + 1) * JB)
            ut = up.tile([P, NS, JB], BF16, tag="ut", name=f"ut_{J}")
            uts[J] = ut
            for sp in range(J + 1):
                si0 = 2 * sp
                ps_t = pst.tile([P, 2 * JB], F32, tag="pst",
                                name=f"pst_{J}_{sp}")
                if sp < J:
                    for h in range(2):
                        si = si0 + h
                        for ct in range(2):
                            nc.tensor.matmul(
                                ps_t[:, h * JB:(h + 1) * JB],
                                qkT["k"][:, ct, si * P:(si + 1) * P],
                                qkT["q"][:, ct, jt],
                                start=(ct == 0), stop=(ct == 1),
                            )
                    nc.scalar.activation(ut[:, si0:si0 + 2, :], ps_t[:],
                                         mybir.ActivationFunctionType.Exp)
                else:
                    # diagonal pair: si0 needs all 256 t-cols; si0+1 only
                    # its second 128 (AV q=0 never reads si0+1) -> N=128
                    for ct in range(2):
                        nc.tensor.matmul(
                            ps_t[:, 0:JB],
                            qkT["k"][:, ct, si0 * P:(si0 + 1) * P],
                            qkT["q"][:, ct, jt],
                            start=(ct == 0), stop=(ct == 1),
                        )
                    for ct in range(2):
                        nc.tensor.matmul(
                            ps_t[:, JB:JB + P],
                            qkT["k"][:, ct, (si0 + 1) * P:(si0 + 2) * P],
                            qkT["q"][:, ct, J * JB + P:(J + 1) * JB],
                            start=(ct == 0), stop=(ct == 1),
                        )
                    nc.scalar.activation(ut[:, si0, :], ps_t[:, 0:JB],
                                         mybir.ActivationFunctionType.Exp)
                    nc.scalar.activation(ut[:, si0 + 1, P:JB],
                                         ps_t[:, JB:JB + P],
                                         mybir.ActivationFunctionType.Exp)
                    nc.vector.tensor_tensor(ut[:, si0, :], ut[:, si0, :],
                                            msk_t[:, 0, :],
                                            mybir.AluOpType.mult)
                    nc.vector.tensor_tensor(ut[:, si0 + 1, P:JB],
                                            ut[:, si0 + 1, P:JB],
                                            msk_t[:, 1, P:JB],
                                            mybir.AluOpType.mult)

        def dr_group(ps, mat, cols, psl, w_stationary=True):
            # 3-term compensated fp8 projection: x1W1 + x1W2 + x2W1.
            # w_stationary: True -> psum [c, t] (Q/K transposed layout);
            # False -> psum [t, c] (V layout).
            first = True
            for (xt, wh) in ((xtr1, 1), (xtr1, 2), (xtr2, 1)):
                for dp in range(NDP):
                    wap = wr[mat, wh][:, 2 * dp:2 * dp + 2, cols]
                    xap = xt[:, 2 * dp:2 * dp + 2, psl]
                    nc.tensor.matmul(
                        ps,
                        wap if w_stationary else xap,
                        xap if w_stationary else wap,
                        start=first, stop=(xt is xtr2 and dp == NDP - 1),
                        perf_mode=DR,
                    )
                    first = False

        def do_v(tb):
            # V projection for chunk tb's two s-tiles
            for si in (2 * tb, 2 * tb + 1):
                pv = pav.tile([P, C + 1], F32, tag="pav", name=f"pv_{si}")[:, :JB]
                dr_group(pv, "v", slice(0, C), slice(si * P, (si + 1) * P),
                         w_stationary=False)
                nc.vector.tensor_tensor(vg[:, si, :C], pv, bvb_t[:],
                                        mybir.AluOpType.add)

        def do_av(J):
            ut = uts[J]
            for q in range(2):
                tci = 2 * J + q
                pa = pav.tile([P, C + 1], F32, tag="pav", name=f"pav_{J}_{q}")
                for si in range(tci + 1):
                    nc.tensor.matmul(
                        pa[:],
                        ut[:, si, q * P:(q + 1) * P],
                        vg[:, si, :],
                        start=(si == 0), stop=(si == tci),
                    )
                recip = outp.tile([P, 1], F32, tag="recip")
                nc.vector.reciprocal(recip[:], pa[:, C:C + 1])
                ob = outp.tile([P, C], F32, tag="ob")
                nc.vector.tensor_scalar_mul(ob[:], pa[:, :C], recip[:])
                nc.sync.dma_start(o[tci * P:(tci + 1) * P, :], ob[:])

        def do_proj(psl):
            pw = psl.stop - psl.start
            for mi, (mat, dsc) in enumerate((("q", QSC), ("k", KSC))):
                for ct in range(2):
                    ps_p = pp.tile([P, 2 * JB], F32, tag="pp",
                                   name=f"pp_{mat}{ct}_{psl.start}")[:, :pw]
                    dr_group(ps_p, mat, slice(ct * P, (ct + 1) * P), psl)
                    nc.vector.tensor_scalar(
                        qkT[mat][:, ct, psl],
                        ps_p,
                        dsc,
                        bqk_t[:, 2 * mi + ct: 2 * mi + ct + 1],
                        mybir.AluOpType.mult,
                        mybir.AluOpType.add,
                    )

        # ---- pair 0: startup DMAs, projections, scores only (V/AV for
        # chunks 0-1 are deferred into pair 1 so the DMA queue delivers
        # x chunks 2-3 before the V weights).  Term order in dr_group is
        # (x1,w1),(x1,w2),(x2,w1) so x2/wq2 may trail x1/wq1 slightly. ----
        ts0 = slice(0, JB)
        x1src0 = x1T[:, ts0].rearrange("(o p) t -> p o t", p=P)
        x2src0 = x2T[:, ts0].rearrange("(o p) t -> p o t", p=P)
        wq1_src = wsrc["q", 1].rearrange("(o p) c -> p o c", p=P)
        wq2_src = wsrc["q", 2].rearrange("(o p) c -> p o c", p=P)
        nc.sync.dma_start(wr["q", 1][:, :, 0:P], wq1_src[:, :, 0:P])
        nc.sync.dma_start(xtr1[:, :, ts0], x1src0)
        nc.sync.dma_start(wr["q", 2][:, :, 0:P], wq2_src[:, :, 0:P])
        nc.sync.dma_start(xtr2[:, :, ts0], x2src0)
        nc.sync.dma_start(wr["q", 1][:, :, P:C], wq1_src[:, :, P:C])
        nc.sync.dma_start(wr["q", 2][:, :, P:C], wq2_src[:, :, P:C])
        nc.sync.dma_start(wr["k", 1][:],
                          wsrc["k", 1].rearrange("(o p) c -> p o c", p=P))
        nc.sync.dma_start(wr["k", 2][:],
                          wsrc["k", 2].rearrange("(o p) c -> p o c", p=P))
        ts1 = slice(JB, 2 * JB)
        nc.sync.dma_start(xtr1[:, :, ts1],
                          x1T[:, ts1].rearrange("(o p) t -> p o t", p=P))
        nc.sync.dma_start(xtr2[:, :, ts1],
                          x2T[:, ts1].rearrange("(o p) t -> p o t", p=P))
        nc.sync.dma_start(msk_t[:], msk)
        nc.sync.dma_start(bvb_t[:], bvb)
        do_proj(ts0)
        do_proj(ts1)
        do_st(0)
        do_st(1)

        # ---- pair 1: x chunks 2-3 queue ahead of the V weights; then the
        # deferred V/AV for chunks 0-1 (emitted after the wv DMA so the RAW
        # dependency is tracked and properly semaphore-guarded) ----
        pts = slice(2 * JB, 4 * JB)
        nc.sync.dma_start(xtr1[:, :, pts],
                          x1T[:, pts].rearrange("(o p) t -> p o t", p=P))
        nc.sync.dma_start(xtr2[:, :, pts],
                          x2T[:, pts].rearrange("(o p) t -> p o t", p=P))
        nc.sync.dma_start(wr["v", 1][:],
                          wsrc["v", 1].rearrange("(o p) c -> p o c", p=P))
        nc.sync.dma_start(wr["v", 2][:],
                          wsrc["v", 2].rearrange("(o p) c -> p o c", p=P))
        do_v(0)
        do_av(0)
        do_v(1)
        do_av(1)
        do_proj(pts)
        for tb in (2, 3):
            do_st(tb)
            do_v(tb)
            do_av(tb)

        # ---- pairs 2-3 ----
        for pb in (2, 3):
            tb0 = 2 * pb
            pts = slice(tb0 * JB, (tb0 + 2) * JB)
            nc.sync.dma_start(xtr1[:, :, pts],
                              x1T[:, pts].rearrange("(o p) t -> p o t", p=P))
            nc.sync.dma_start(xtr2[:, :, pts],
                              x2T[:, pts].rearrange("(o p) t -> p o t", p=P))
            do_proj(pts)
            for tb in (tb0, tb0 + 1):
                do_st(tb)
                do_v(tb)
                do_av(tb)

    nc.compile()
    return nc


_CACHE = {}
LAST_EXEC_TIME_NS = None


def _get_nc():
    if "nc" not in _CACHE:
        _CACHE["nc"] = _build()
    return _CACHE["nc"]


E4 = ml_dtypes.float8_e4m3


def _split8(a, scale):
    a1 = (a * scale).astype(E4)
    a2 = (a * scale - a1.astype(np.float32)).astype(E4)
    return a1, a2


def kernel(x, Wq, bq, Wk, bk, Wv, bv):
    x = np.asarray(x, dtype=np.float32)
    Wq = np.asarray(Wq, dtype=np.float32)
    bq = np.asarray(bq, dtype=np.float32)
    Wk = np.asarray(Wk, dtype=np.float32)
    bk = np.asarray(bk, dtype=np.float32)
    Wv = np.asarray(Wv, dtype=np.float32)
    bv = np.asarray(bv, dtype=np.float32)

    # Fold the 4x head-tiling into the weights: contraction with tile(Kg,4)
    # equals contraction of block-summed Q with Kg.
    wq_s = Wq.reshape(D, 4, C).sum(axis=1, dtype=np.float64).astype(np.float32)
    bq_s = bq.reshape(4, C).sum(axis=0, dtype=np.float64).astype(np.float32)

    bqk = np.stack([bq_s[:P], bq_s[P:], bk[:P], bk[P:]], axis=1).astype(np.float32)
    bvb = np.broadcast_to(VSC * bv, (P, C)).astype(np.float32)

    # Diagonal-block causal masks: keep t >= s  <=>  j >= 128*m + p.
    jj = np.arange(JB)[None, None, :]
    pp_ = np.arange(P)[:, None, None]
    mm = np.arange(2)[None, :, None]
    msk = (jj >= P * mm + pp_).astype(ml_dtypes.float8_e4m3)

    wq1, wq2 = _split8(wq_s, SWQ)
    wk1, wk2 = _split8(Wk, SWK)
    wv1, wv2 = _split8(Wv, SWK)
    shared = {
        "wq1": np.ascontiguousarray(wq1), "wq2": np.ascontiguousarray(wq2),
        "wk1": np.ascontiguousarray(wk1), "wk2": np.ascontiguousarray(wk2),
        "wv1": np.ascontiguousarray(wv1), "wv2": np.ascontiguousarray(wv2),
        "bqk": np.ascontiguousarray(bqk),
        "bvb": np.ascontiguousarray(bvb.astype(np.float16)),
        "msk": np.ascontiguousarray(msk),
    }
    in_maps = []
    for b in range(B):
        x1, x2 = _split8(x[b].T, SX)
        m = dict(shared)
        m["x1T"] = np.ascontiguousarray(x1)
        m["x2T"] = np.ascontiguousarray(x2)
        in_maps.append(m)

    nc = _get_nc()
    try:
        res = run_bass_kernel_spmd(nc, in_maps, core_ids=list(range(NCORES)))
    except ModuleNotFoundError:
        # BASS_TRACE=1 requests NTFF profiling, but this container type has
        # no axon NTFF hook (antenv.axon_hooks absent) -- rerun untraced.
        os.environ["BASS_NEVER_TRACE"] = "1"
        res = run_bass_kernel_spmd(nc, in_maps, core_ids=list(range(NCORES)))
    global LAST_EXEC_TIME_NS
    LAST_EXEC_TIME_NS = res.exec_time_ns
    if res.exec_time_ns is not None:
        print(f"HW exec time: {res.exec_time_ns} ns")

    out = np.empty((1, B, T, 4 * C), dtype=np.float32)
    for b in range(B):
        ob = res.results[b]["o"]
        out[0, b] = np.tile(ob, (1, 4))
    return out


# revision 9
# speedup vs baseline: 1.1302x; 1.0609x over previous
"""Causal attention (flattened-head GQA variant) for TRN2, 8 NeuronCores.

Problem structure exploited:
  - K/V are group-projections tiled 4x along the head dim, and the score
    contraction runs over the full flattened 1024 dim.  Algebraically:
        att = Q @ tile(Kg,4)^T = (sum of Q's four 256-col blocks) @ Kg^T
        out = att_sm @ tile(Vg,4) = tile(att_sm @ Vg, 4)
    so the device only computes with 256-wide Qsum/Kg/Vg.
  - Projections run in fp8e4m3 DoubleRow mode (PE contracts 256 rows/pass at
    0.5 cycles/row = 4x fp16 rate) with error compensation: host splits
    x ~ x1 + x2 and W ~ W1 + W2 (each fp8, residual split), device computes
    x1W1 + x1W2 + x2W1 in one PSUM group (12 DR matmuls vs 16 fp16-equiv
    passes).  Dropped x2W2 term ~2^-8 relative.  Operands are pre-scaled by
    powers of 2 (x: 32, W: 4096 / 1024 for the Q block-sum) to sit in e4m3's
    normal range; the PSUM scale (2^15 Q / 2^17 K,V) is absorbed by the fused
    scale+bias tensor_scalar for Q/K, and for V rides into vg where it
    cancels against the ones-column (8*2^17) in the rowsum normalization.
  - Softmax needs no max-subtraction (logits bounded ~60; exp fits fp32),
    so scores are computed directly in the transposed layout
    U^T[s,t] = exp(Kg @ Qsum^T) and fed straight into the AV matmul as the
    stationary operand -- no on-device transposes at all.
  - Block-causal skipping: s-tiles entirely above the diagonal are never
    computed; diagonal 128x256 blocks are masked with precomputed 0/1 tiles.
  - Fused chunk loop keeps PE saturated while DMA streams x chunks.

Precision: fp8-3-term projections (~2^-8 rel), score matmul fp16, exp/AV
path bf16.  End-to-end absmax rel error vs fp32 reference ~1.75e-2.

Sharding: data-parallel over batch B=8, one batch per core, no collectives.
"""

import os
import numpy as np
import ml_dtypes
from contextlib import ExitStack

import concourse.tile as tile
from concourse import bacc, mybir
from concourse.bass_utils import run_bass_kernel_spmd

B, T, D = 8, 2048, 1024
C = 256          # group width (N_QUERY_GROUPS * HEAD_SIZE)
P = 128
ND = D // P      # 8 contraction tiles for projections
NDP = ND // 2    # 4 DoubleRow contraction pairs
NS = T // P      # 16 s-tiles
JB = 256         # t-chunk width
NJB = T // JB    # 8
NCORES = 8

F32 = mybir.dt.float32
FP16 = mybir.dt.float16
BF16 = mybir.dt.bfloat16
FP8 = mybir.dt.float8e4
DR = mybir.MatmulPerfMode.DoubleRow

SX = 32.0        # x pre-scale into e4m3 range
SWQ = 1024.0     # wq_s pre-scale (block-summed W spans +-0.125)
SWK = 4096.0     # wk/wv pre-scale (+-1/32)
QSC = 1.0 / (SX * SWQ)   # PSUM -> Q descale
KSC = 1.0 / (SX * SWK)   # PSUM -> K descale
VSC = SX * SWK           # V path stays scaled; ones-col = 8*VSC


def _build():
    nc = bacc.Bacc("TRN2", target_bir_lowering=False, debug=False)
    x1T = nc.dram_tensor("x1T", [D, T], FP8, kind="ExternalInput").ap()
    x2T = nc.dram_tensor("x2T", [D, T], FP8, kind="ExternalInput").ap()
    # w1|w2 stacked side-by-side: 512B contiguous rows dodge the <512B
    # 2x DMA latency multiplier.
    wsrc = {}
    for _n in ("q", "k", "v"):
        wsrc[_n] = nc.dram_tensor(f"w{_n}", [D, 2 * C], FP8,
                                  kind="ExternalInput").ap()
    bqk = nc.dram_tensor("bqk", [P, 4], F32, kind="ExternalInput").ap()
    bvb = nc.dram_tensor("bvb", [P, C], FP16, kind="ExternalInput").ap()
    msk = nc.dram_tensor("msk", [P, 2, JB], mybir.dt.float8e4, kind="ExternalInput").ap()
    o = nc.dram_tensor("o", [T, C], F32, kind="ExternalOutput").ap()

    with tile.TileContext(nc) as tc, ExitStack() as ctx:
        cst = ctx.enter_context(tc.tile_pool(name="cst", bufs=1))
        big = ctx.enter_context(tc.tile_pool(name="big", bufs=1))
        up = ctx.enter_context(tc.tile_pool(name="up", bufs=3))
        outp = ctx.enter_context(tc.tile_pool(name="outp", bufs=3))
        pp = ctx.enter_context(tc.tile_pool(name="pp", bufs=2, space="PSUM"))
        pst = ctx.enter_context(tc.tile_pool(name="pst", bufs=4, space="PSUM"))
        pav = ctx.enter_context(tc.tile_pool(name="pav", bufs=2, space="PSUM"))

        bqk_t = cst.tile([P, 4], F32, tag="bqk")
        nc.sync.dma_start(bqk_t[:], bqk)
        bvb_t = cst.tile([P, C], FP16, tag="bvb")
        msk_t = cst.tile([P, 2, JB], mybir.dt.float8e4, tag="msk")

        wr12 = {}
        wr = {}
        for _n in ("q", "k", "v"):
            wr12[_n] = cst.tile([P, ND, 2 * C], FP8, tag=f"w{_n}",
                                name=f"wr_{_n}")
            wr[_n, 1] = wr12[_n][:, :, 0:C]
            wr[_n, 2] = wr12[_n][:, :, C:2 * C]

        # PE warm-up: ~5us of matmuls on a zeroed scratch tile while the
        # first DMAs land -- fills the idle start window and finishes the
        # PE clock ramp (HAM) before real work arrives.
        wrm = cst.tile([P, C], FP16, tag="wrm")
        nc.vector.memset(wrm[:], 0.0)
        for wi in range(16):
            ps_w = pp.tile([P, 2 * JB], F32, tag="pp", name=f"warm_{wi}")
            nc.tensor.matmul(ps_w[:, :C], wrm[:, :P], wrm[:], start=True, stop=True)

        xtr1 = big.tile([P, ND, T], FP8, tag="xtr1")
        xtr2 = big.tile([P, ND, T], FP8, tag="xtr2")
        qkT = {"q": big.tile([P, 2, T], FP16, tag="qsT", name="qsT"),
               "k": big.tile([P, 2, T], FP16, tag="ksT", name="ksT")}
        vg = big.tile([P, NS, C + 1], BF16, tag="vg")
        nc.vector.memset(vg[:, :, C:C + 1], 8.0 * VSC)

        uts = {}

        def do_st(J):
            # scores^T -> exp for t-block J
            jt = slice(J * JB, (J + 1) * JB)
            ut = up.tile([P, NS, JB], BF16, tag="ut", name=f"ut_{J}")
            uts[J] = ut
            for sp in range(J + 1):
                si0 = 2 * sp
                ps_t = pst.tile([P, 2 * JB], F32, tag="pst",
                                name=f"pst_{J}_{sp}")
                if sp < J:
                    for h in range(2):
                        si = si0 + h
                        for ct in range(2):
                            nc.tensor.matmul(
                                ps_t[:, h * JB:(h + 1) * JB],
                                qkT["k"][:, ct, si * P:(si + 1) * P],
                                qkT["q"][:, ct, jt],
                                start=(ct == 0), stop=(ct == 1),
                            )
                    nc.scalar.activation(ut[:, si0:si0 + 2, :], ps_t[:],
                                         mybir.ActivationFunctionType.Exp)
                else:
                    # diagonal pair: si0 needs all 256 t-cols; si0+1 only
                    # its second 128 (AV q=0 never reads si0+1) -> N=128
                    for ct in range(2):
                        nc.tensor.matmul(
                            ps_t[:, 0:JB],
                            qkT["k"][:, ct, si0 * P:(si0 + 1) * P],
                            qkT["q"][:, ct, jt],
                            start=(ct == 0), stop=(ct == 1),
                        )
                    for ct in range(2):
                        nc.tensor.matmul(
                            ps_t[:, JB:JB + P],
                            qkT["k"][:, ct, (si0 + 1) * P:(si0 + 2) * P],
                            qkT["q"][:, ct, J * JB + P:(J + 1) * JB],
                            start=(ct == 0), stop=(ct == 1),
                        )
                    nc.scalar.activation(ut[:, si0, :], ps_t[:, 0:JB],
                                         mybir.ActivationFunctionType.Exp)
                    nc.scalar.activation(ut[:, si0 + 1, P:JB],
                                         ps_t[:, JB:JB + P],
                                         mybir.ActivationFunctionType.Exp)
                    nc.vector.tensor_tensor(ut[:, si0, :], ut[:, si0, :],
                                            msk_t[:, 0, :],
                                            mybir.AluOpType.mult)
                    nc.vector.tensor_tensor(ut[:, si0 + 1, P:JB],
                                            ut[:, si0 + 1, P:JB],
                                            msk_t[:, 1, P:JB],
                                            mybir.AluOpType.mult)

        def dr_group(ps, mat, cols, psl, w_stationary=True):
            # 3-term compensated fp8 projection: x1W1 + x1W2 + x2W1.
            # w_stationary: True -> psum [c, t] (Q/K transposed layout);
            # False -> psum [t, c] (V layout).
            first = True
            for (xt, wh) in ((xtr1, 1), (xtr1, 2), (xtr2, 1)):
                for dp in range(NDP):
                    wap = wr[mat, wh][:, 2 * dp:2 * dp + 2, cols]
                    xap = xt[:, 2 * dp:2 * dp + 2, psl]
                    nc.tensor.matmul(
                        ps,
                        wap if w_stationary else xap,
                        xap if w_stationary else wap,
                        start=first, stop=(xt is xtr2 and dp == NDP - 1),
                        perf_mode=DR,
                    )
                    first = False

        def do_v(tb):
            # V projection for chunk tb's two s-tiles
            for si in (2 * tb, 2 * tb + 1):
                pv = pav.tile([P, C + 1], F32, tag="pav", name=f"pv_{si}")[:, :JB]
                dr_group(pv, "v", slice(0, C), slice(si * P, (si + 1) * P),
                         w_stationary=False)
                nc.vector.tensor_tensor(vg[:, si, :C], pv, bvb_t[:],
                                        mybir.AluOpType.add)

        def do_av(J):
            ut = uts[J]
            for q in range(2):
                tci = 2 * J + q
                pa = pav.tile([P, C + 1], F32, tag="pav", name=f"pav_{J}_{q}")
                for si in range(tci + 1):
                    nc.tensor.matmul(
                        pa[:],
                        ut[:, si, q * P:(q + 1) * P],
                        vg[:, si, :],
                        start=(si == 0), stop=(si == tci),
                    )
                recip = outp.tile([P, 1], F32, tag="recip")
                nc.vector.reciprocal(recip[:], pa[:, C:C + 1])
                ob = outp.tile([P, C], F32, tag="ob")
                nc.vector.tensor_scalar_mul(ob[:], pa[:, :C], recip[:])
                nc.sync.dma_start(o[tci * P:(tci + 1) * P, :], ob[:])

        def do_proj(psl):
            pw = psl.stop - psl.start
            for mi, (mat, dsc) in enumerate((("q", QSC), ("k", KSC))):
                for ct in range(2):
                    ps_p = pp.tile([P, 2 * JB], F32, tag="pp",
                                   name=f"pp_{mat}{ct}_{psl.start}")[:, :pw]
                    dr_group(ps_p, mat, slice(ct * P, (ct + 1) * P), psl)
                    nc.vector.tensor_scalar(
                        qkT[mat][:, ct, psl],
                        ps_p,
                        dsc,
                        bqk_t[:, 2 * mi + ct: 2 * mi + ct + 1],
                        mybir.AluOpType.mult,
                        mybir.AluOpType.add,
                    )

        def do_proj_split(psl):
            # Startup variant: emit all x1-terms (8 DR per group) for the 4
            # Q/K groups first, then the x2-terms, then biases -- the PE can
            # start as soon as x1+w arrive, with x2 still in flight.
            pw = psl.stop - psl.start
            tiles = {}
            for mat in ("q", "k"):
                for ct in range(2):
                    ps_p = pst.tile([P, 2 * JB], F32, tag="pst",
                                    name=f"pps_{mat}{ct}_{psl.start}")[:, :pw]
                    tiles[mat, ct] = ps_p
                    first = True
                    for wh in (1, 2):
                        for dp in range(NDP):
                            nc.tensor.matmul(
                                ps_p,
                                wr[mat, wh][:, 2 * dp:2 * dp + 2,
                                            ct * P:(ct + 1) * P],
                                xtr1[:, 2 * dp:2 * dp + 2, psl],
                                start=first, stop=False,
                                perf_mode=DR,
                            )
                            first = False
            for mi, (mat, dsc) in enumerate((("q", QSC), ("k", KSC))):
                for ct in range(2):
                    ps_p = tiles[mat, ct]
                    for dp in range(NDP):
                        nc.tensor.matmul(
                            ps_p,
                            wr[mat, 1][:, 2 * dp:2 * dp + 2,
                                       ct * P:(ct + 1) * P],
                            xtr2[:, 2 * dp:2 * dp + 2, psl],
                            start=False, stop=(dp == NDP - 1),
                            perf_mode=DR,
                        )
                    nc.vector.tensor_scalar(
                        qkT[mat][:, ct, psl],
                        ps_p,
                        dsc,
                        bqk_t[:, 2 * mi + ct: 2 * mi + ct + 1],
                        mybir.AluOpType.mult,
                        mybir.AluOpType.add,
                    )

        # ---- pair 0: startup DMAs, split-phase projections (x1 terms can
        # start while x2 is still in flight), scores only (V/AV for chunks
        # 0-1 are deferred into pair 1 so the DMA queue delivers x pair 1
        # before the V weights). ----
        p0 = slice(0, 2 * JB)
        nc.sync.dma_start(wr12["q"][:],
                          wsrc["q"].rearrange("(o p) c -> p o c", p=P))
        nc.sync.dma_start(xtr1[:, :, p0],
                          x1T[:, p0].rearrange("(o p) t -> p o t", p=P))
        nc.sync.dma_start(wr12["k"][:],
                          wsrc["k"].rearrange("(o p) c -> p o c", p=P))
        nc.sync.dma_start(xtr2[:, :, p0],
                          x2T[:, p0].rearrange("(o p) t -> p o t", p=P))
        nc.sync.dma_start(msk_t[:], msk)
        nc.sync.dma_start(bvb_t[:], bvb)
        ts0 = slice(0, JB)
        ts1 = slice(JB, 2 * JB)
        do_proj_split(ts0)
        do_proj(ts1)
        do_st(0)
        do_st(1)

        # ---- pair 1: x pair 1 queues ahead of the V weights; then the
        # deferred V/AV for chunks 0-1 (emitted after the wv DMA so the RAW
        # dependency is tracked and properly semaphore-guarded) ----
        pts = slice(2 * JB, 4 * JB)
        nc.sync.dma_start(xtr1[:, :, pts],
                          x1T[:, pts].rearrange("(o p) t -> p o t", p=P))
        nc.sync.dma_start(xtr2[:, :, pts],
                          x2T[:, pts].rearrange("(o p) t -> p o t", p=P))
        nc.sync.dma_start(wr12["v"][:],
                          wsrc["v"].rearrange("(o p) c -> p o c", p=P))
        do_v(0)
        do_av(0)
        do_v(1)
        do_av(1)
        do_proj(pts)
        for tb in (2, 3):
            do_st(tb)
            do_v(tb)
            do_av(tb)

        # ---- pairs 2-3 ----
        for pb in (2, 3):
            tb0 = 2 * pb
            pts = slice(tb0 * JB, (tb0 + 2) * JB)
            nc.sync.dma_start(xtr1[:, :, pts],
                              x1T[:, pts].rearrange("(o p) t -> p o t", p=P))
            nc.sync.dma_start(xtr2[:, :, pts],
                              x2T[:, pts].rearrange("(o p) t -> p o t", p=P))
            do_proj(pts)
            for tb in (tb0, tb0 + 1):
                do_st(tb)
                do_v(tb)
                do_av(tb)

    nc.compile()
    return nc


_CACHE = {}
LAST_EXEC_TIME_NS = None


def _get_nc():
    if "nc" not in _CACHE:
        _CACHE["nc"] = _build()
    return _CACHE["nc"]


E4 = ml_dtypes.float8_e4m3


def _split8(a, scale):
    a1 = (a * scale).astype(E4)
    a2 = (a * scale - a1.astype(np.float32)).astype(E4)
    return a1, a2


def kernel(x, Wq, bq, Wk, bk, Wv, bv):
    x = np.asarray(x, dtype=np.float32)
    Wq = np.asarray(Wq, dtype=np.float32)
    bq = np.asarray(bq, dtype=np.float32)
    Wk = np.asarray(Wk, dtype=np.float32)
    bk = np.asarray(bk, dtype=np.float32)
    Wv = np.asarray(Wv, dtype=np.float32)
    bv = np.asarray(bv, dtype=np.float32)

    # Fold the 4x head-tiling into the weights: contraction with tile(Kg,4)
    # equals contraction of block-summed Q with Kg.
    wq_s = Wq.reshape(D, 4, C).sum(axis=1, dtype=np.float64).astype(np.float32)
    bq_s = bq.reshape(4, C).sum(axis=0, dtype=np.float64).astype(np.float32)

    bqk = np.stack([bq_s[:P], bq_s[P:], bk[:P], bk[P:]], axis=1).astype(np.float32)
    bvb = np.broadcast_to(VSC * bv, (P, C)).astype(np.float32)

    # Diagonal-block causal masks: keep t >= s  <=>  j >= 128*m + p.
    jj = np.arange(JB)[None, None, :]
    pp_ = np.arange(P)[:, None, None]
    mm = np.arange(2)[None, :, None]
    msk = (jj >= P * mm + pp_).astype(ml_dtypes.float8_e4m3)

    shared = {
        "wq": np.ascontiguousarray(np.concatenate(_split8(wq_s, SWQ), axis=1)),
        "wk": np.ascontiguousarray(np.concatenate(_split8(Wk, SWK), axis=1)),
        "wv": np.ascontiguousarray(np.concatenate(_split8(Wv, SWK), axis=1)),
        "bqk": np.ascontiguousarray(bqk),
        "bvb": np.ascontiguousarray(bvb.astype(np.float16)),
        "msk": np.ascontiguousarray(msk),
    }
    in_maps = []
    for b in range(B):
        x1, x2 = _split8(x[b].T, SX)
        m = dict(shared)
        m["x1T"] = np.ascontiguousarray(x1)
        m["x2T"] = np.ascontiguousarray(x2)
        in_maps.append(m)

    nc = _get_nc()
    try:
        res = run_bass_kernel_spmd(nc, in_maps, core_ids=list(range(NCORES)))
    except ModuleNotFoundError:
        # BASS_TRACE=1 requests NTFF profiling, but this container type has
        # no axon NTFF hook (antenv.axon_hooks absent) -- rerun untraced.
        os.environ["BASS_NEVER_TRACE"] = "1"
        res = run_bass_kernel_spmd(nc, in_maps, core_ids=list(range(NCORES)))
    global LAST_EXEC_TIME_NS
    LAST_EXEC_TIME_NS = res.exec_time_ns
    if res.exec_time_ns is not None:
        print(f"HW exec time: {res.exec_time_ns} ns")

    out = np.empty((1, B, T, 4 * C), dtype=np.float32)
    for b in range(B):
        ob = res.results[b]["o"]
        out[0, b] = np.tile(ob, (1, 4))
    return out


# revision 13
# speedup vs baseline: 1.1441x; 1.0123x over previous
"""Causal attention (flattened-head GQA variant) for TRN2, 8 NeuronCores.

Problem structure exploited:
  - K/V are group-projections tiled 4x along the head dim, and the score
    contraction runs over the full flattened 1024 dim.  Algebraically:
        att = Q @ tile(Kg,4)^T = (sum of Q's four 256-col blocks) @ Kg^T
        out = att_sm @ tile(Vg,4) = tile(att_sm @ Vg, 4)
    so the device only computes with 256-wide Qsum/Kg/Vg.
  - Projections run in fp8e4m3 DoubleRow mode (PE contracts 256 rows/pass at
    0.5 cycles/row = 4x fp16 rate) with error compensation: host splits
    x ~ x1 + x2 and W ~ W1 + W2 (each fp8, residual split), device computes
    x1W1 + x1W2 + x2W1 in one PSUM group (12 DR matmuls vs 16 fp16-equiv
    passes).  Dropped x2W2 term ~2^-8 relative.  Operands are pre-scaled by
    powers of 2 (x: 32, W: 4096 / 1024 for the Q block-sum) to sit in e4m3's
    normal range; the PSUM scale (2^15 Q / 2^17 K,V) is absorbed by the fused
    scale+bias tensor_scalar for Q/K, and for V rides into vg where it
    cancels against the ones-column (8*2^17) in the rowsum normalization.
  - Softmax needs no max-subtraction (logits bounded ~60; exp fits fp32),
    so scores are computed directly in the transposed layout
    U^T[s,t] = exp(Kg @ Qsum^T) and fed straight into the AV matmul as the
    stationary operand -- no on-device transposes at all.
  - Block-causal skipping: s-tiles entirely above the diagonal are never
    computed; diagonal 128x256 blocks are masked with precomputed 0/1 tiles.
  - Fused chunk loop keeps PE saturated while DMA streams x chunks.

Precision: fp8-3-term projections (~2^-8 rel), score matmul fp16, exp/AV
path bf16.  End-to-end absmax rel error vs fp32 reference ~1.75e-2.

Sharding: data-parallel over batch B=8, one batch per core, no collectives.
"""

import os
import numpy as np
import ml_dtypes
from contextlib import ExitStack

import concourse.tile as tile
from concourse import bacc, mybir
from concourse.bass_utils import run_bass_kernel_spmd

B, T, D = 8, 2048, 1024
C = 256          # group width (N_QUERY_GROUPS * HEAD_SIZE)
P = 128
ND = D // P      # 8 contraction tiles for projections
NDP = ND // 2    # 4 DoubleRow contraction pairs
NS = T // P      # 16 s-tiles
JB = 256         # t-chunk width
NJB = T // JB    # 8
NCORES = 8

F32 = mybir.dt.float32
FP16 = mybir.dt.float16
BF16 = mybir.dt.bfloat16
FP8 = mybir.dt.float8e4
DR = mybir.MatmulPerfMode.DoubleRow

SX = 32.0        # x pre-scale into e4m3 range
SWQ = 1024.0     # wq_s pre-scale (block-summed W spans +-0.125)
SWK = 4096.0     # wk/wv pre-scale (+-1/32)
QSC = 1.0 / (SX * SWQ)   # PSUM -> Q descale
KSC = 1.0 / (SX * SWK)   # PSUM -> K descale
VSC = SX * SWK           # V path stays scaled; ones-col = 8*VSC


def _build():
    nc = bacc.Bacc("TRN2", target_bir_lowering=False, debug=False)
    x1T = nc.dram_tensor("x1T", [D, T], FP8, kind="ExternalInput").ap()
    x2T = nc.dram_tensor("x2T", [D, T], FP8, kind="ExternalInput").ap()
    # w1|w2 stacked side-by-side: 512B contiguous rows dodge the <512B
    # 2x DMA latency multiplier.
    wsrc = {}
    for _n in ("q", "k", "v"):
        wsrc[_n] = nc.dram_tensor(f"w{_n}", [D, 2 * C], FP8,
                                  kind="ExternalInput").ap()
    bqk = nc.dram_tensor("bqk", [P, 4], F32, kind="ExternalInput").ap()
    bvb = nc.dram_tensor("bvb", [P, C], FP16, kind="ExternalInput").ap()
    msk = nc.dram_tensor("msk", [P, 2, JB], mybir.dt.float8e4, kind="ExternalInput").ap()
    o = nc.dram_tensor("o", [T, C], F32, kind="ExternalOutput").ap()

    with tile.TileContext(nc) as tc, ExitStack() as ctx:
        cst = ctx.enter_context(tc.tile_pool(name="cst", bufs=1))
        big = ctx.enter_context(tc.tile_pool(name="big", bufs=1))
        up = ctx.enter_context(tc.tile_pool(name="up", bufs=3))
        outp = ctx.enter_context(tc.tile_pool(name="outp", bufs=3))
        pp = ctx.enter_context(tc.tile_pool(name="pp", bufs=2, space="PSUM"))
        pst = ctx.enter_context(tc.tile_pool(name="pst", bufs=4, space="PSUM"))
        pav = ctx.enter_context(tc.tile_pool(name="pav", bufs=2, space="PSUM"))

        bqk_t = cst.tile([P, 4], F32, tag="bqk")
        nc.sync.dma_start(bqk_t[:], bqk)
        bvb_t = cst.tile([P, C], FP16, tag="bvb")
        msk_t = cst.tile([P, 2, JB], mybir.dt.float8e4, tag="msk")

        wr12 = {}
        wr = {}
        for _n in ("q", "k", "v"):
            wr12[_n] = cst.tile([P, ND, 2 * C], FP8, tag=f"w{_n}",
                                name=f"wr_{_n}")
            wr[_n, 1] = wr12[_n][:, :, 0:C]
            wr[_n, 2] = wr12[_n][:, :, C:2 * C]

        # PE warm-up: ~5us of matmuls on a zeroed scratch tile while the
        # first DMAs land -- fills the idle start window and finishes the
        # PE clock ramp (HAM) before real work arrives.
        wrm = cst.tile([P, C], FP16, tag="wrm")
        nc.vector.memset(wrm[:], 0.0)
        for wi in range(16):
            ps_w = pp.tile([P, 2 * JB], F32, tag="pp", name=f"warm_{wi}")
            nc.tensor.matmul(ps_w[:, :C], wrm[:, :P], wrm[:], start=True, stop=True)

        xtr1 = big.tile([P, ND, T], FP8, tag="xtr1")
        xtr2 = big.tile([P, ND, T], FP8, tag="xtr2")
        qkT = {"q": big.tile([P, 2, T], FP16, tag="qsT", name="qsT"),
               "k": big.tile([P, 2, T], FP16, tag="ksT", name="ksT")}
        vg = big.tile([P, NS, C + 1], BF16, tag="vg")
        nc.vector.memset(vg[:, :, C:C + 1], 8.0 * VSC)

        uts = {}

        def do_st(J):
            # scores^T -> exp for t-block J
            jt = slice(J * JB, (J + 1) * JB)
            ut = up.tile([P, NS, JB], BF16, tag="ut", name=f"ut_{J}")
            uts[J] = ut
            for sp in range(J + 1):
                si0 = 2 * sp
                ps_t = pst.tile([P, 2 * JB], F32, tag="pst",
                                name=f"pst_{J}_{sp}")
                if sp < J:
                    for h in range(2):
                        si = si0 + h
                        for ct in range(2):
                            nc.tensor.matmul(
                                ps_t[:, h * JB:(h + 1) * JB],
                                qkT["k"][:, ct, si * P:(si + 1) * P],
                                qkT["q"][:, ct, jt],
                                start=(ct == 0), stop=(ct == 1),
                            )
                    nc.scalar.activation(ut[:, si0:si0 + 2, :], ps_t[:],
                                         mybir.ActivationFunctionType.Exp)
                else:
                    # diagonal pair: si0 needs all 256 t-cols; si0+1 only
                    # its second 128 (AV q=0 never reads si0+1) -> N=128
                    for ct in range(2):
                        nc.tensor.matmul(
                            ps_t[:, 0:JB],
                            qkT["k"][:, ct, si0 * P:(si0 + 1) * P],
                            qkT["q"][:, ct, jt],
                            start=(ct == 0), stop=(ct == 1),
                        )
                    for ct in range(2):
                        nc.tensor.matmul(
                            ps_t[:, JB:JB + P],
                            qkT["k"][:, ct, (si0 + 1) * P:(si0 + 2) * P],
                            qkT["q"][:, ct, J * JB + P:(J + 1) * JB],
                            start=(ct == 0), stop=(ct == 1),
                        )
                    nc.scalar.activation(ut[:, si0, :], ps_t[:, 0:JB],
                                         mybir.ActivationFunctionType.Exp)
                    nc.scalar.activation(ut[:, si0 + 1, P:JB],
                                         ps_t[:, JB:JB + P],
                                         mybir.ActivationFunctionType.Exp)
                    nc.vector.tensor_tensor(ut[:, si0, :], ut[:, si0, :],
                                            msk_t[:, 0, :],
                                            mybir.AluOpType.mult)
                    nc.vector.tensor_tensor(ut[:, si0 + 1, P:JB],
                                            ut[:, si0 + 1, P:JB],
                                            msk_t[:, 1, P:JB],
                                            mybir.AluOpType.mult)

        def dr_group(ps, mat, cols, psl, w_stationary=True):
            # 3-term compensated fp8 projection: x1W1 + x1W2 + x2W1.
            # w_stationary: True -> psum [c, t] (Q/K transposed layout);
            # False -> psum [t, c] (V layout).
            first = True
            for (xt, wh) in ((xtr1, 1), (xtr1, 2), (xtr2, 1)):
                for dp in range(NDP):
                    wap = wr[mat, wh][:, 2 * dp:2 * dp + 2, cols]
                    xap = xt[:, 2 * dp:2 * dp + 2, psl]
                    nc.tensor.matmul(
                        ps,
                        wap if w_stationary else xap,
                        xap if w_stationary else wap,
                        start=first, stop=(xt is xtr2 and dp == NDP - 1),
                        perf_mode=DR,
                    )
                    first = False

        def do_v(tb):
            # V projection for chunk tb's two s-tiles
            for si in (2 * tb, 2 * tb + 1):
                pv = pav.tile([P, C + 1], F32, tag="pav", name=f"pv_{si}")[:, :JB]
                dr_group(pv, "v", slice(0, C), slice(si * P, (si + 1) * P),
                         w_stationary=False)
                nc.vector.tensor_tensor(vg[:, si, :C], pv, bvb_t[:],
                                        mybir.AluOpType.add)

        def do_av(J):
            ut = uts[J]
            for q in range(2):
                tci = 2 * J + q
                pa = pav.tile([P, C + 1], F32, tag="pav", name=f"pav_{J}_{q}")
                for si in range(tci + 1):
                    nc.tensor.matmul(
                        pa[:],
                        ut[:, si, q * P:(q + 1) * P],
                        vg[:, si, :],
                        start=(si == 0), stop=(si == tci),
                    )
                recip = outp.tile([P, 1], F32, tag="recip")
                nc.vector.reciprocal(recip[:], pa[:, C:C + 1])
                ob = outp.tile([P, C], F32, tag="ob")
                nc.vector.tensor_scalar_mul(ob[:], pa[:, :C], recip[:])
                nc.sync.dma_start(o[tci * P:(tci + 1) * P, :], ob[:])

        def do_proj(psl):
            pw = psl.stop - psl.start
            for mi, (mat, dsc) in enumerate((("q", QSC), ("k", KSC))):
                for ct in range(2):
                    ps_p = pp.tile([P, 2 * JB], F32, tag="pp",
                                   name=f"pp_{mat}{ct}_{psl.start}")[:, :pw]
                    dr_group(ps_p, mat, slice(ct * P, (ct + 1) * P), psl)
                    nc.vector.tensor_scalar(
                        qkT[mat][:, ct, psl],
                        ps_p,
                        dsc,
                        bqk_t[:, 2 * mi + ct: 2 * mi + ct + 1],
                        mybir.AluOpType.mult,
                        mybir.AluOpType.add,
                    )

        def proj_x1_phase(psl, tiles, pools):
            # Startup variant, phase 1: emit all x1-terms (8 DR per group)
            # for the 4 Q/K groups -- the PE can start as soon as x1+w
            # arrive, with x2 still in flight.  Groups stay open in `tiles`.
            pw = psl.stop - psl.start
            for gi, (mat, ct) in enumerate(
                    (("q", 0), ("q", 1), ("k", 0), ("k", 1))):
                pool, shape, tag = pools[gi]
                ps_p = pool.tile(shape, F32, tag=tag,
                                 name=f"pps_{mat}{ct}_{psl.start}")[:, :pw]
                tiles[mat, ct] = ps_p
                first = True
                for wh in (1, 2):
                    for dp in range(NDP):
                        nc.tensor.matmul(
                            ps_p,
                            wr[mat, wh][:, 2 * dp:2 * dp + 2,
                                        ct * P:(ct + 1) * P],
                            xtr1[:, 2 * dp:2 * dp + 2, psl],
                            start=first, stop=False,
                            perf_mode=DR,
                        )
                        first = False

        def proj_x2_phase(psl, tiles):
            # Startup variant, phase 2: x2-terms close the groups; biases.
            for mi, (mat, dsc) in enumerate((("q", QSC), ("k", KSC))):
                for ct in range(2):
                    ps_p = tiles[mat, ct]
                    for dp in range(NDP):
                        nc.tensor.matmul(
                            ps_p,
                            wr[mat, 1][:, 2 * dp:2 * dp + 2,
                                       ct * P:(ct + 1) * P],
                            xtr2[:, 2 * dp:2 * dp + 2, psl],
                            start=False, stop=(dp == NDP - 1),
                            perf_mode=DR,
                        )
                    nc.vector.tensor_scalar(
                        qkT[mat][:, ct, psl],
                        ps_p,
                        dsc,
                        bqk_t[:, 2 * mi + ct: 2 * mi + ct + 1],
                        mybir.AluOpType.mult,
                        mybir.AluOpType.add,
                    )

        # ---- pair 0: startup DMAs, split-phase projections (x1 terms can
        # start while x2 is still in flight), scores only (V/AV for chunks
        # 0-1 are deferred into pair 1 so the DMA queue delivers x pair 1
        # before the V weights). ----
        p0 = slice(0, 2 * JB)
        nc.sync.dma_start(wr12["q"][:],
                          wsrc["q"].rearrange("(o p) c -> p o c", p=P))
        nc.sync.dma_start(xtr1[:, :, p0],
                          x1T[:, p0].rearrange("(o p) t -> p o t", p=P))
        nc.sync.dma_start(wr12["k"][:],
                          wsrc["k"].rearrange("(o p) c -> p o c", p=P))
        nc.sync.dma_start(xtr2[:, :, p0],
                          x2T[:, p0].rearrange("(o p) t -> p o t", p=P))
        nc.sync.dma_start(msk_t[:], msk)
        nc.sync.dma_start(bvb_t[:], bvb)
        ts0 = slice(0, JB)
        ts1 = slice(JB, 2 * JB)
        # x1-phases for BOTH chunks bridge the PE until x2 pair 0 lands;
        # 8 held PSUM groups: 4 from pst, 2 from pp, 2 from pav.
        t0_tiles, t1_tiles = {}, {}
        proj_x1_phase(ts0, t0_tiles, [(pst, [P, 2 * JB], "pst")] * 4)
        proj_x1_phase(ts1, t1_tiles,
                      [(pp, [P, 2 * JB], "pp"), (pp, [P, 2 * JB], "pp"),
                       (pav, [P, C + 1], "pav"), (pav, [P, C + 1], "pav")])
        proj_x2_phase(ts0, t0_tiles)
        proj_x2_phase(ts1, t1_tiles)
        do_st(0)
        do_st(1)

        # ---- pair 1: x pair 1 queues ahead of the V weights; then the
        # deferred V/AV for chunks 0-1 (emitted after the wv DMA so the RAW
        # dependency is tracked and properly semaphore-guarded) ----
        pts = slice(2 * JB, 4 * JB)
        nc.sync.dma_start(xtr1[:, :, pts],
                          x1T[:, pts].rearrange("(o p) t -> p o t", p=P))
        nc.sync.dma_start(xtr2[:, :, pts],
                          x2T[:, pts].rearrange("(o p) t -> p o t", p=P))
        nc.sync.dma_start(wr12["v"][:],
                          wsrc["v"].rearrange("(o p) c -> p o c", p=P))
        do_v(0)
        do_av(0)
        do_v(1)
        do_av(1)
        do_proj(pts)
        for tb in (2, 3):
            do_st(tb)
            do_v(tb)
            do_av(tb)

        # ---- pairs 2-3 ----
        for pb in (2, 3):
            tb0 = 2 * pb
            pts = slice(tb0 * JB, (tb0 + 2) * JB)
            nc.sync.dma_start(xtr1[:, :, pts],
                              x1T[:, pts].rearrange("(o p) t -> p o t", p=P))
            nc.sync.dma_start(xtr2[:, :, pts],
                              x2T[:, pts].rearrange("(o p) t -> p o t", p=P))
            do_proj(pts)
            for tb in (tb0, tb0 + 1):
                do_st(tb)
                do_v(tb)
                do_av(tb)

    nc.compile()
    return nc


_CACHE = {}
LAST_EXEC_TIME_NS = None


def _get_nc():
    if "nc" not in _CACHE:
        _CACHE["nc"] = _build()
    return _CACHE["nc"]


E4 = ml_dtypes.float8_e4m3


def _split8(a, scale):
    a1 = (a * scale).astype(E4)
    a2 = (a * scale - a1.astype(np.float32)).astype(E4)
    return a1, a2


def kernel(x, Wq, bq, Wk, bk, Wv, bv):
    x = np.asarray(x, dtype=np.float32)
    Wq = np.asarray(Wq, dtype=np.float32)
    bq = np.asarray(bq, dtype=np.float32)
    Wk = np.asarray(Wk, dtype=np.float32)
    bk = np.asarray(bk, dtype=np.float32)
    Wv = np.asarray(Wv, dtype=np.float32)
    bv = np.asarray(bv, dtype=np.float32)

    # Fold the 4x head-tiling into the weights: contraction with tile(Kg,4)
    # equals contraction of block-summed Q with Kg.
    wq_s = Wq.reshape(D, 4, C).sum(axis=1, dtype=np.float64).astype(np.float32)
    bq_s = bq.reshape(4, C).sum(axis=0, dtype=np.float64).astype(np.float32)

    bqk = np.stack([bq_s[:P], bq_s[P:], bk[:P], bk[P:]], axis=1).astype(np.float32)
    bvb = np.broadcast_to(VSC * bv, (P, C)).astype(np.float32)

    # Diagonal-block causal masks: keep t >= s  <=>  j >= 128*m + p.
    jj = np.arange(JB)[None, None, :]
    pp_ = np.arange(P)[:, None, None]
    mm = np.arange(2)[None, :, None]
    msk = (jj >= P * mm + pp_).astype(ml_dtypes.float8_e4m3)

    shared = {
        "wq": np.ascontiguousarray(np.concatenate(_split8(wq_s, SWQ), axis=1)),
        "wk": np.ascontiguousarray(np.concatenate(_split8(Wk, SWK), axis=1)),
        "wv": np.ascontiguousarray(np.concatenate(_split8(Wv, SWK), axis=1)),
        "bqk": np.ascontiguousarray(bqk),
        "bvb": np.ascontiguousarray(bvb.astype(np.float16)),
        "msk": np.ascontiguousarray(msk),
    }
    in_maps = []
    for b in range(B):
        x1, x2 = _split8(x[b].T, SX)
        m = dict(shared)
        m["x1T"] = np.ascontiguousarray(x1)
        m["x2T"] = np.ascontiguousarray(x2)
        in_maps.append(m)

    nc = _get_nc()
    try:
        res = run_bass_kernel_spmd(nc, in_maps, core_ids=list(range(NCORES)))
    except ModuleNotFoundError:
        # BASS_TRACE=1 requests NTFF profiling, but this container type has
        # no axon NTFF hook (antenv.axon_hooks absent) -- rerun untraced.
        os.environ["BASS_NEVER_TRACE"] = "1"
        res = run_bass_kernel_spmd(nc, in_maps, core_ids=list(range(NCORES)))
    global LAST_EXEC_TIME_NS
    LAST_EXEC_TIME_NS = res.exec_time_ns
    if res.exec_time_ns is not None:
        print(f"HW exec time: {res.exec_time_ns} ns")

    out = np.empty((1, B, T, 4 * C), dtype=np.float32)
    for b in range(B):
        ob = res.results[b]["o"]
        out[0, b] = np.tile(ob, (1, 4))
    return out
